# revision 1
# baseline (speedup 1.0000x reference)
"""Trainium2 Bass kernel for nn_CenterAgent (scatter_memory).

Self-contained: takes FULL inputs (B=256), shards batch across 8 NeuronCores
(pure data parallel, 32 samples/core), runs one Bass/Tile program per core via
run_bass_kernel_spmd, gathers the full [256, 24] output.

v2 structure (per core):
  Phase A — scatter on 128 partitions (4 blocks of 25 centers per sample on
    the partition dim) + stage0 (the 512->128 channel contraction of conv1 at
    7x7) for all 32 samples up front, M-packed 2 samples per matmul and run
    as fp8e4m3 DoubleRow over kb-block pairs.  PE never waits on the scatter.
  Phase B — conv1 = im2col (image+cmap, fp8 DR) + upsample-sum U matmuls
    (bf16, 420-wide padded rows so conv windows are contiguous); conv2/3/4 as
    fp8e4m3 DoubleRow tap-pair matmuls against [2copy, 32, 30] activation
    buffers where copy1 is a one-row-shifted DMA dup (vertical tap pairs
    (t, t+3) share one instruction); sample packing across the shrinking
    channel dims via staging + partition-shift DMAs.
  Tail — fc1 as fp8 DR over (col-block, ch-pair) k-tiles, fc2 f32r.
"""

import os
from contextlib import ExitStack

import ml_dtypes
import numpy as np

import concourse.bass as bass
import concourse.tile as tile
from concourse import bacc, mybir
from concourse.bass_utils import run_bass_kernel_spmd

NCORES = 8
B = 256
BL = B // NCORES  # 32 samples per core
SC = 8            # samples per pipeline chunk
F32 = mybir.dt.float32
F32R = mybir.dt.float32r
F16 = mybir.dt.float16
BF16 = mybir.dt.bfloat16
F8 = mybir.dt.float8e4
I16 = mybir.dt.int16
U8 = mybir.dt.uint8
ALU = mybir.AluOpType
ACTF = mybir.ActivationFunctionType
DR = mybir.MatmulPerfMode.DoubleRow

EVEN = [0, 2, 4, 6]
ODD = [1, 3, 5, 7]
# conv tap-pair slots: (t, t+3) vertical pairs share a DoubleRow instruction
# (copy1 = one-row-shifted dup); taps 6,7,8 run as singles w/ a zero 2nd tile.
PAIRS = [(0, 3), (1, 4), (2, 5), (6, None), (7, None), (8, None)]

f8 = ml_dtypes.float8_e4m3


# ----------------------------------------------------------------- host consts

def _bilinear_A():
    A = np.zeros((28, 7), np.float32)
    for i in range(28):
        t = (i + 0.5) / 4 - 0.5
        p0 = int(np.floor(t))
        w = t - p0
        A[i, min(max(p0, 0), 6)] += 1 - w
        A[i, min(max(p0 + 1, 0), 6)] += w
    return A


def _shifted_A(d):
    A = _bilinear_A()
    S = np.zeros_like(A)
    for i in range(28):
        src = i + d - 1
        if 0 <= src < 28:
            S[i] = A[src]
    return S


def _utap(t):
    di, dj = divmod(t, 3)
    return np.einsum(
        "ip,jq->pqij", _shifted_A(di), _shifted_A(dj)
    ).reshape(49, 784).astype(np.float32)


def _pad840(u):
    # [49, 784] -> [49, 840]: embed 28-wide rows into 30-wide (junk cols 0, 29)
    out = np.zeros((49, 2, 14, 30), np.float32)
    u4 = u.reshape(49, 2, 14, 28)
    out[:, :, :, 1:29] = u4
    return out.reshape(49, 840)


def _wpair(wt_list):
    # wt_list: per tap t -> [K, M] f32; build [K, 6, 2, M] fp8 pair const
    K, M = wt_list[0].shape
    out = np.zeros((K, 6, 2, M), np.float32)
    for si, (t0, t1) in enumerate(PAIRS):
        out[:, si, 0, :] = wt_list[t0]
        if t1 is not None:
            out[:, si, 1, :] = wt_list[t1]
    return out.astype(f8)


def _build_consts(w1, b1, w2, b2, w3, b3, w4, b4, fw1, fb1, fw2, fb2):
    w1 = np.asarray(w1, np.float32)
    w1f = w1[:, 3:515]  # [128o, 512c, 3, 3]
    c_w1fe = np.zeros((128, 4, 512), np.float32)
    c_w1fo = np.zeros((128, 4, 512), np.float32)
    c_w1f8 = np.zeros((128, 4, 128), np.float32)
    for kb in range(4):
        blk = w1f[:, kb * 128:(kb + 1) * 128]  # [o, c, 3, 3]
        for ti, t in enumerate(EVEN):
            di, dj = divmod(t, 3)
            c_w1fe[:, kb, ti * 128:(ti + 1) * 128] = blk[:, :, di, dj].T
        for ti, t in enumerate(ODD):
            di, dj = divmod(t, 3)
            c_w1fo[:, kb, ti * 128:(ti + 1) * 128] = blk[:, :, di, dj].T
        c_w1f8[:, kb, :] = blk[:, :, 2, 2].T

    # c_uall [128, 5, 840] bf16: rows 0-48 even taps / U8, rows 49-97 odd taps
    c_uall = np.zeros((128, 5, 840), np.float32)
    for t in range(4):
        c_uall[0:49, t] = _pad840(_utap(EVEN[t]))
        c_uall[49:98, t] = _pad840(_utap(ODD[t]))
    c_uall[0:49, 4] = _pad840(_utap(8))

    # image+cmap im2col weights: K rows = (di, ch), fp8, dj k-tile pairs
    chmap = [0, 1, 2, 515]
    c_w1ic = np.zeros((12, 2, 2, 128), np.float32)
    for di in range(3):
        for ch in range(4):
            c_w1ic[di * 4 + ch, 0, 0] = w1[:, chmap[ch], di, 0]
            c_w1ic[di * 4 + ch, 0, 1] = w1[:, chmap[ch], di, 1]
            c_w1ic[di * 4 + ch, 1, 0] = w1[:, chmap[ch], di, 2]
    c_w1ic = c_w1ic.astype(f8)

    w2 = np.asarray(w2, np.float32)
    c_w2p = _wpair([w2[:, :, t // 3, t % 3].T for t in range(9)])

    w3 = np.asarray(w3, np.float32)
    w3t = []
    for t in range(9):
        wt = np.zeros((128, 64), np.float32)
        blk = w3[:, :, t // 3, t % 3].T  # [64c, 32o]
        wt[0:64, 0:32] = blk
        wt[64:128, 32:64] = blk
        w3t.append(wt)
    c_w3p = _wpair(w3t)

    w4 = np.asarray(w4, np.float32)
    w4t = []
    for t in range(9):
        wt = np.zeros((128, 64), np.float32)
        blk = w4[:, :, t // 3, t % 3].T  # [32c, 16o]
        for bi in range(4):
            wt[32 * bi:32 * bi + 32, 16 * bi:16 * bi + 16] = blk
        w4t.append(wt)
    c_w4p = _wpair(w4t)

    # fc1: [112, 7, 8chpair, 2, 256] fp8
    f3 = np.asarray(fw1, np.float32).reshape(256, 16, 784)
    c_fw1p = np.zeros((112, 7, 8, 2, 256), np.float32)
    for c in range(7):
        blk = f3[:, :, 112 * c:112 * (c + 1)].transpose(2, 1, 0)  # [112,16,256]
        c_fw1p[:, c] = blk.reshape(112, 8, 2, 256)
    c_fw1p = c_fw1p.astype(f8)

    fw2 = np.asarray(fw2, np.float32)  # [24, 256]
    c_fw2 = np.zeros((128, 2, 24), np.float32)
    c_fw2[:, 0] = fw2[:, 0:128].T
    c_fw2[:, 1] = fw2[:, 128:256].T

    ident2 = np.zeros((128, 64), np.float32)
    ident2[0:64] = np.eye(64)
    ident2[64:128] = np.eye(64)

    # scatter consts on 128 partitions: partition (b, s) = b*32 + s
    tri2 = np.zeros((4, 25, 100), np.float32)
    for b in range(4):
        for i in range(25):
            tri2[b, i, 25 * b + i + 1:] = 1.0
    c_tri2 = np.broadcast_to(tri2[:, None], (4, 32, 25, 100)).reshape(128, 25, 100)

    return {
        "c_w1fe": c_w1fe.astype(f8),
        "c_w1fo": c_w1fo.astype(f8),
        "c_w1f8": c_w1f8.astype(f8),
        "c_uall": c_uall.astype(ml_dtypes.bfloat16),
        "c_w1ic": c_w1ic,
        "c_w2p": c_w2p,
        "c_w3p": c_w3p,
        "c_w4p": c_w4p,
        "c_b1": np.asarray(b1, np.float32).reshape(128, 1),
        "c_b2": np.tile(np.asarray(b2, np.float32), 2).reshape(128, 1),
        "c_b3": np.tile(np.asarray(b3, np.float32), 4).reshape(128, 1),
        "c_b4": np.tile(np.asarray(b4, np.float32), 8).reshape(128, 1),
        "c_fw1p": c_fw1p,
        "c_fb1": np.asarray(fb1, np.float32).reshape(1, 256),
        "c_fw2": c_fw2,
        "c_fb2": np.asarray(fb2, np.float32).reshape(1, 24),
        "c_ident": ident2,
        "c_tri2": np.ascontiguousarray(c_tri2).astype(np.float16),
        "c_k27": np.broadcast_to(np.arange(1, 28, dtype=np.float32),
                                 (128, 27)).copy(),
    }


_CONST_SPECS = {
    "c_w1fe": ([128, 4, 512], F8),
    "c_w1fo": ([128, 4, 512], F8),
    "c_w1f8": ([128, 4, 128], F8),
    "c_uall": ([128, 5, 840], BF16),
    "c_w1ic": ([12, 2, 2, 128], F8),
    "c_w2p": ([128, 6, 2, 64], F8),
    "c_w3p": ([128, 6, 2, 64], F8),
    "c_w4p": ([128, 6, 2, 64], F8),
    "c_b1": ([128, 1], F32),
    "c_b2": ([128, 1], F32),
    "c_b3": ([128, 1], F32),
    "c_b4": ([128, 1], F32),
    "c_fw1p": ([112, 7, 8, 2, 256], F8),
    "c_fb1": ([1, 256], F32R),
    "c_fw2": ([128, 2, 24], F32R),
    "c_fb2": ([1, 24], F32R),
    "c_ident": ([128, 64], F32),
    "c_tri2": ([128, 25, 100], F16),
    "c_k27": ([128, 27], F32),
}


def _pair_ap(buf_ap, base, stride, n=420):
    """[K, 2, n] moving AP at flat free offset `base` within a per-partition
    flat layout; k-tile 1 at base+stride.  buf_ap must be a [K, ...] AP whose
    free dims are contiguous from its own offset."""
    ap_list = [tuple(e) for e in buf_ap.ap]
    return bass.AP(buf_ap.tensor, buf_ap.offset + base,
                   [ap_list[0], (stride, 2), (1, n)])


def _win_base(h, di, dj):
    # flat offset of the contiguous [14, 30] window in a [32, 30] slot
    return (1 + 14 * h + di) * 30 + dj - 1


# ------------------------------------------------------------------ device IR


def build_nc():
    nc = bacc.Bacc("TRN2", target_bir_lowering=False, debug=False)
    image = nc.dram_tensor("image", [BL, 3, 28, 28], F32R, kind="ExternalInput").ap()
    features = nc.dram_tensor("features", [BL, 512, 7, 7], F32R, kind="ExternalInput").ap()
    centers = nc.dram_tensor("centers", [BL, 100, 4], F32, kind="ExternalInput").ap()
    cst = {
        name: nc.dram_tensor(name, shape, dt, kind="ExternalInput").ap()
        for name, (shape, dt) in _CONST_SPECS.items()
    }
    out_d = nc.dram_tensor("out", [BL, 24], F32, kind="ExternalOutput").ap()
    DBG = bool(os.environ.get("KDBG"))
    dbg = {}
    if DBG:
        for nm, shape, dt in [("d_hs0", [128, 640], BF16),
                              ("d_hs1", [128, 640], BF16),
                              ("d_col0", [12, 1800], F8),
                              ("d_x1", [128, 1920], F8),
                              ("d_x2", [128, 1920], F8),
                              ("d_x3", [128, 1920], F8),
                              ("d_x4t", [128, 784], F32),
                              ("d_scr", [4, 900], F8),
                              ("d_idx", [32, 100], I16),
                              ("d_conf", [32, 100], F16),
                              ("d_flat", [128, 25], F32),
                              ("d_later", [128, 25], F16)]:
            dbg[nm] = nc.dram_tensor(nm, shape, dt, kind="ExternalOutput").ap()
    scratch = nc.dram_tensor("scratch", [BL, 4, 30, 30], F8, kind="Internal").ap()

    with tile.TileContext(nc) as tc, ExitStack() as ctx:
        # ------------------------------------------------ constant tiles
        cp = ctx.enter_context(tc.tile_pool(name="consts", bufs=1))
        ct = {}
        for name, (shape, dt) in _CONST_SPECS.items():
            if name in ("c_tri2", "c_k27", "c_fw1p"):
                continue
            ct[name] = cp.tile(shape, dt, tag=name, name=name)
        ones32 = cp.tile([1, 32], F32R, tag="ones32")
        nc.vector.memset(ones32[:].bitcast(F32), 1.0)
        for name in ("c_w1fe", "c_w1fo", "c_w1f8"):
            nc.sync.dma_start(out=ct[name][:], in_=cst[name])

        # ------------------------------------------------ persistent buffers
        pp = ctx.enter_context(tc.tile_pool(name="persist", bufs=1))
        fbuf = pp.tile([128, 16, 4, 112], F8, tag="fbuf")       # 2smp/group
        x1buf = pp.tile([128, SC, 2, 32, 30], F8, tag="x1buf")
        x2buf = pp.tile([128, 4, 2, 32, 30], F8, tag="x2buf")
        x3buf = pp.tile([128, 2, 2, 32, 30], F8, tag="x3buf")
        x4t = pp.tile([128, 2, 392], F32, tag="x4t")
        x4T = pp.tile([112, 7, 8, 2, 32], F8, tag="x4T")
        nc.gpsimd.memset(x1buf[:].bitcast(U8), 0)
        nc.gpsimd.memset(x2buf[:].bitcast(U8), 0)
        nc.gpsimd.memset(x3buf[:].bitcast(U8), 0)

        hp = ctx.enter_context(tc.tile_pool(name="hbuf", bufs=1))
        Hs = [hp.tile([128, 640], BF16, tag=f"H{s}", name=f"Hs{s}")
              for s in range(BL)]
        # zero rows beyond the data regions (PE computes NaN*0=NaN, so
        # uninitialized rows under zero U coefficients would still poison PSUM)
        for s in range(BL):
            nc.gpsimd.memset(Hs[s][:], 0.0)

        fwp = ctx.enter_context(tc.tile_pool(name="fw1", bufs=1))
        c_fw1_t = fwp.tile([112, 7, 8, 2, 256], F8, tag="c_fw1", name="c_fw1_t")

        # ---------------------------------------------- scatter (128-part)
        with tc.tile_pool(name="scat", bufs=1) as sp:
            tri2 = sp.tile([128, 25, 100], F16, tag="tri2")
            nc.sync.dma_start(out=tri2[:], in_=cst["c_tri2"])
            cen4 = sp.tile([128, 25, 4], F32, tag="cen4")
            for b in range(4):
                nc.sync.dma_start(out=cen4[32 * b:32 * b + 32, :, :],
                                  in_=centers[:, 25 * b:25 * b + 25, :])
            k27 = sp.tile([128, 27], F32, tag="k27")
            nc.sync.dma_start(out=k27[:], in_=cst["c_k27"])

            ge = sp.tile([128, 25, 27], F32, tag="ge")

            def floor28(dst, coord_ap, name):
                v = sp.tile([128, 25], F32, tag=name, name=name)
                nc.vector.tensor_scalar_mul(v[:], coord_ap, 28.0)
                nc.vector.tensor_tensor(
                    ge[:],
                    v[:].unsqueeze(2).broadcast_to([128, 25, 27]),
                    k27[:].unsqueeze(1).broadcast_to([128, 25, 27]),
                    ALU.is_ge,
                )
                nc.vector.tensor_reduce(dst[:], ge[:], mybir.AxisListType.X,
                                        ALU.add)

            xp = sp.tile([128, 25], F32, tag="xp")
            floor28(xp, cen4[:, :, 0], "xs")
            yp = sp.tile([128, 25], F32, tag="yp")
            floor28(yp, cen4[:, :, 1], "ys")
            flat = sp.tile([128, 25], F32, tag="flat")
            nc.vector.scalar_tensor_tensor(flat[:], yp[:], 30.0, xp[:],
                                           ALU.mult, ALU.add)
            nc.vector.tensor_scalar_add(flat[:], flat[:], 31.0)
            flat16 = sp.tile([128, 25], F16, tag="flat16")
            nc.vector.tensor_copy(flat16[:], flat[:])

            flat_s = sp.tile([32, 100], F16, tag="flat_s")
            for b in range(4):
                nc.sync.dma_start(out=flat_s[:, 25 * b:25 * b + 25],
                                  in_=flat16[32 * b:32 * b + 32, :])
            flatAll = sp.tile([128, 100], F16, tag="flatAll")
            for b in range(4):
                nc.sync.dma_start(out=flatAll[32 * b:32 * b + 32, :],
                                  in_=flat_s[:])

            D = sp.tile([128, 25, 100], F16, tag="D")
            nc.vector.tensor_tensor(
                D[:],
                flat16[:].unsqueeze(2).broadcast_to([128, 25, 100]),
                flatAll[:].unsqueeze(1).broadcast_to([128, 25, 100]),
                ALU.is_equal)
            E = sp.tile([128, 25, 100], F16, tag="E")
            nc.vector.tensor_mul(E[:], D[:], tri2[:])
            later = sp.tile([128, 25], F16, tag="later")
            nc.vector.tensor_reduce(later[:], E[:], mybir.AxisListType.X,
                                    ALU.max)
            lateri = sp.tile([128, 25], U8, tag="lateri")
            nc.vector.tensor_copy(lateri[:], later[:])
            neg1 = sp.tile([128, 25], F32, tag="neg1")
            nc.vector.memset(neg1[:], -1.0)
            idxf = sp.tile([128, 25], F32, tag="idxf")
            nc.vector.select(idxf[:], lateri[:], neg1[:], flat[:])
            idx16p = sp.tile([128, 25], I16, tag="idx16p")
            nc.vector.tensor_copy(idx16p[:], idxf[:])
            conf16p = sp.tile([128, 25], F16, tag="conf16p")
            nc.vector.tensor_copy(conf16p[:], cen4[:, :, 3])

            idx_s = sp.tile([32, 100], I16, tag="idx_s")
            conf_s = sp.tile([32, 100], F16, tag="conf_s")
            for b in range(4):
                nc.sync.dma_start(out=idx_s[:, 25 * b:25 * b + 25],
                                  in_=idx16p[32 * b:32 * b + 32, :])
                nc.sync.dma_start(out=conf_s[:, 25 * b:25 * b + 25],
                                  in_=conf16p[32 * b:32 * b + 32, :])

            if DBG:
                nc.sync.dma_start(out=dbg["d_idx"], in_=idx_s[:])
                nc.sync.dma_start(out=dbg["d_conf"], in_=conf_s[:])
                nc.sync.dma_start(out=dbg["d_flat"], in_=flat[:])
                nc.sync.dma_start(out=dbg["d_later"], in_=later[:])
            cmap16 = sp.tile([32, 900], F16, tag="cmap16")
            nc.gpsimd.local_scatter(cmap16[:], conf_s[:], idx_s[:],
                                    channels=32, num_elems=900, num_idxs=100)
            cmap8 = sp.tile([32, 900], F8, tag="cmap8")
            nc.vector.tensor_copy(cmap8[:], cmap16[:])
            nc.sync.dma_start(
                out=scratch[:, 3].rearrange("s a b -> s (a b)"),
                in_=cmap8[:])

            # image zero-pad straight to fp8 scratch
            ipad = sp.tile([96, 30, 30], F8, tag="ipad")
            nc.gpsimd.memset(ipad[:].bitcast(U8), 0)
            nc.gpsimd.dma_start(out=ipad[:, 1:29, 1:29],
                                in_=image.rearrange("s c h w -> (s c) h w"))
            nc.sync.dma_start(out=scratch[:, 0:3], in_=ipad[:])

        # remaining consts
        for name in ct:
            if name not in ("c_w1fe", "c_w1fo", "c_w1f8"):
                nc.sync.dma_start(out=ct[name][:], in_=cst[name])

        # ---------------------------------------------- phase A: stage0 x32
        sgp = ctx.enter_context(tc.tile_pool(name="s0stg", bufs=3))
        with tc.tile_pool(name="psA", bufs=4, space="PSUM") as psA:
            for g in range(16):
                for half in range(2):
                    s = 2 * g + half
                    nc.gpsimd.dma_start(
                        out=fbuf[:, g, :, 49 * half:49 * half + 49],
                        in_=features[s].rearrange("(k c) h w -> c k (h w)", k=4),
                    )
                psE = psA.tile([128, 2, 512], F32, tag="ps", name="psE")
                psO = psA.tile([128, 2, 512], F32, tag="ps", name="psO")
                for ki in range(2):
                    lhs = fbuf[:, g, 2 * ki:2 * ki + 2, 0:98]
                    nc.tensor.matmul(
                        psE[0:98, 0, :], lhs,
                        ct["c_w1fe"][:, 2 * ki:2 * ki + 2, :],
                        start=(ki == 0), stop=(ki == 1), perf_mode=DR)
                    nc.tensor.matmul(
                        psO[0:98, 0, :], lhs,
                        ct["c_w1fo"][:, 2 * ki:2 * ki + 2, :],
                        start=(ki == 0), stop=(ki == 1), perf_mode=DR)
                    nc.tensor.matmul(
                        psE[0:98, 1, 0:128], lhs,
                        ct["c_w1f8"][:, 2 * ki:2 * ki + 2, :],
                        start=(ki == 0), stop=(ki == 1), perf_mode=DR)
                # staging (bf16) then partition-shift DMAs into Hs tiles
                sE = sgp.tile([98, 640], BF16, tag="sE", name="sE")
                sO = sgp.tile([98, 512], BF16, tag="sO", name="sO")
                nc.scalar.copy(
                    sE[:], psE[0:98, :, :].rearrange("p a b -> p (a b)")[:, 0:640])
                nc.scalar.copy(sO[:], psO[0:98, 0, :])
                hA, hB = Hs[2 * g], Hs[2 * g + 1]
                # sample A: even+tap8 -> rows 0-48, odd -> rows 49-97
                nc.sync.dma_start(out=hA[0:49, :], in_=sE[0:49, :])
                nc.sync.dma_start(out=hA[49:98, 0:512], in_=sO[0:49, :])
                # sample B: even+tap8 -> rows 0-48, odd -> rows 49-97
                nc.sync.dma_start(out=hB[0:49, :], in_=sE[49:98, :])
                nc.sync.dma_start(out=hB[49:98, 0:512], in_=sO[49:98, :])

        if DBG:
            nc.sync.dma_start(out=dbg["d_hs0"], in_=Hs[0][:])
            nc.sync.dma_start(out=dbg["d_hs1"], in_=Hs[1][:])
            nc.sync.dma_start(out=dbg["d_scr"],
                              in_=scratch[0].rearrange("c a b -> c (a b)"))

        # ---------------------------------------------- phase B: chunks
        ps1p = ctx.enter_context(tc.tile_pool(name="ps1", bufs=2, space="PSUM"))
        pcv = ctx.enter_context(tc.tile_pool(name="pcv", bufs=4, space="PSUM"))
        colp = ctx.enter_context(tc.tile_pool(name="col", bufs=4))
        stgp = ctx.enter_context(tc.tile_pool(name="stg", bufs=2))

        def conv_dr(ps_out, wconst, xbuf, slot, h):
            # 6 DoubleRow instrs accumulating one h-half of a 3x3 conv
            xa = xbuf[:, slot].rearrange("p a b c -> p (a b c)")
            for si_, (t0, _) in enumerate(PAIRS):
                di, dj = divmod(t0, 3)
                nc.tensor.matmul(
                    ps_out, wconst[:, si_, :, :],
                    _pair_ap(xa, _win_base(h, di, dj), 960),
                    start=(si_ == 0), stop=(si_ == 5), perf_mode=DR)

        def act_pad(dst_interior, ps_h, bias):
            # ps_h: [P, 420] psum; write relu into padded interior [P, 14, 28]
            nc.scalar.activation(
                dst_interior,
                ps_h.rearrange("p (a b) -> p a b", a=14)[:, :, 1:29],
                ACTF.Relu, bias=bias, scale=1.0)

        for ci in range(4):
            for si in range(SC):
                s = ci * SC + si
                # im2col rows: col30 [12, 2copy, 30, 30] fp8
                col30 = colp.tile([12, 2, 30, 30], F8, tag="col30")
                for di in range(3):
                    nc.sync.dma_start(
                        out=col30[4 * di:4 * di + 4, 0, 1:29, :],
                        in_=scratch[s, :, di:di + 28, :],
                    )
                cf = col30[:].rearrange("p a b c -> p (a b c)")
                nc.sync.dma_start(out=cf[:, 900:1799], in_=cf[:, 1:900])

                # h-slots padded to 512 floats so each accumulation group
                # stays inside one 2KB PSUM bank
                ps1 = ps1p.tile([128, 2, 512], F32, tag="ps1")
                for h in range(2):
                    o_ap = ps1[:, h, 0:420]
                    # im2col fp8 DR: pair (dj0, dj1) then single dj2
                    nc.tensor.matmul(
                        o_ap, ct["c_w1ic"][:, 0, :, :],
                        _pair_ap(cf, _win_base(h, 0, 0), 900),
                        start=True, stop=False, perf_mode=DR)
                    nc.tensor.matmul(
                        o_ap, ct["c_w1ic"][:, 1, :, :],
                        _pair_ap(cf, _win_base(h, 0, 2), 900),
                        start=False, stop=False, perf_mode=DR)
                    for t in range(5):
                        nc.tensor.matmul(
                            o_ap,
                            Hs[s][:, 128 * t:128 * (t + 1)],
                            ct["c_uall"][:, t, h * 420:(h + 1) * 420],
                            start=False, stop=(t == 4))
                for h in range(2):
                    act_pad(x1buf[:, si, 0, 2 + 14 * h:16 + 14 * h, 1:29],
                            ps1[:, h, 0:420], ct["c_b1"][:])
                xf1 = x1buf[:, si].rearrange("p a b c -> p (a b c)")
                nc.sync.dma_start(out=xf1[:, 960:1890], in_=xf1[:, 30:960])
                if DBG and s == 0:
                    nc.sync.dma_start(out=dbg["d_col0"],
                                      in_=col30[:].rearrange("p a b c -> p (a b c)"))
                    nc.sync.dma_start(out=dbg["d_x1"], in_=xf1)

            # conv2: per sample (DR can't col-tile); x2buf packs sample pairs
            for p in range(4):
                for half in range(2):
                    s2 = 2 * p + half
                    stg2 = stgp.tile([64, 28, 28], F8, tag="stg2", name="stg2") if half else None
                    for h in range(2):
                        ps2 = pcv.tile([128, 420], F32, tag="pc", name="ps2")
                        conv_dr(ps2[0:64, :], ct["c_w2p"], x1buf, s2, h)
                        if half == 0:
                            act_pad(x2buf[0:64, p, 0,
                                          2 + 14 * h:16 + 14 * h, 1:29],
                                    ps2[0:64, :], ct["c_b2"][0:64])
                        else:
                            act_pad(stg2[:, 14 * h:14 * h + 14, :],
                                    ps2[0:64, :], ct["c_b2"][0:64])
                    if half == 1:
                        nc.sync.dma_start(
                            out=x2buf[64:128, p, 0, 2:30, 1:29], in_=stg2[:])
                xf2 = x2buf[:, p].rearrange("p a b c -> p (a b c)")
                nc.sync.dma_start(out=xf2[:, 960:1890], in_=xf2[:, 30:960])
                if DBG and ci == 0 and p == 0:
                    nc.sync.dma_start(out=dbg["d_x2"], in_=xf2)

            # conv3: K packs a sample pair; x3buf packs quads
            for q in range(2):
                for half in range(2):
                    pp_ = 2 * q + half
                    stg3 = stgp.tile([64, 28, 28], F8, tag="stg3", name="stg3") if half else None
                    for h in range(2):
                        ps3 = pcv.tile([128, 420], F32, tag="pc", name="ps3")
                        conv_dr(ps3[0:64, :], ct["c_w3p"], x2buf, pp_, h)
                        if half == 0:
                            act_pad(x3buf[0:64, q, 0,
                                          2 + 14 * h:16 + 14 * h, 1:29],
                                    ps3[0:64, :], ct["c_b3"][0:64])
                        else:
                            act_pad(stg3[:, 14 * h:14 * h + 14, :],
                                    ps3[0:64, :], ct["c_b3"][0:64])
                    if half == 1:
                        nc.sync.dma_start(
                            out=x3buf[64:128, q, 0, 2:30, 1:29], in_=stg3[:])
                xf3 = x3buf[:, q].rearrange("p a b c -> p (a b c)")
                nc.sync.dma_start(out=xf3[:, 960:1890], in_=xf3[:, 30:960])
                if DBG and ci == 0 and q == 0:
                    nc.sync.dma_start(out=dbg["d_x3"], in_=xf3)

            # conv4: K packs a quad; x4t [128 = 8smp x 16oc, 2, 392] f32
            for g4 in range(2):
                stg4 = stgp.tile([64, 2, 392], F32, tag="stg4", name="stg4") if g4 else None
                for h in range(2):
                    ps4 = pcv.tile([128, 420], F32, tag="pc", name="ps4")
                    conv_dr(ps4[0:64, :], ct["c_w4p"], x3buf, g4, h)
                    dst = x4t[0:64, h, :] if g4 == 0 else stg4[:, h, :]
                    nc.scalar.activation(
                        dst,
                        ps4[0:64, :].rearrange(
                            "p (a b) -> p a b", a=14)[:, :, 1:29],
                        ACTF.Relu, bias=ct["c_b4"][0:64], scale=1.0)
                if g4 == 1:
                    nc.sync.dma_start(out=x4t[64:128, :, :], in_=stg4[:])

            if DBG and ci == 0:
                nc.sync.dma_start(out=dbg["d_x4t"],
                                  in_=x4t[:].rearrange("p a b -> p (a b)"))

            # transpose x4t -> x4T fp8 [112, 7, 8cp, 2c2, 32smp]
            for h2 in range(2):
                g = ci * 2 + h2
                for c in range(7):
                    tr = pcv.tile([112, 64], F32, tag="pc", name="tr")
                    nc.tensor.transpose(
                        tr[:],
                        x4t[64 * h2:64 * h2 + 64, :, :].rearrange(
                            "p a b -> p (a b)")[:, 112 * c:112 * (c + 1)],
                        ct["c_ident"][64 * h2:64 * h2 + 64, 0:64],
                    )
                    nc.vector.tensor_copy(
                        x4T[:, c, :, :, 4 * g:4 * g + 4],
                        tr[:].rearrange("p (s cp c2) -> p cp c2 s",
                                        s=4, cp=8))

            for c in ([2 * ci, 2 * ci + 1] if ci < 3 else [6]):
                nc.sync.dma_start(out=c_fw1_t[:, c], in_=cst["c_fw1p"][:, c])

        # ------------------------------------------------ fc1 / fc2
        smp = ctx.enter_context(tc.tile_pool(name="small", bufs=2))
        psF = pcv.tile([32, 256], F32, tag="pc", name="psF")
        nc.tensor.matmul(psF[:], ones32[:], ct["c_fb1"][:],
                         start=True, stop=False)
        for c in range(7):
            for cp_ in range(8):
                nc.tensor.matmul(
                    psF[:],
                    x4T[:, c, cp_, :, :],
                    c_fw1_t[:, c, cp_, :, :],
                    start=False, stop=(c == 6 and cp_ == 7), perf_mode=DR)
        x5 = smp.tile([32, 256], F32, tag="x5")
        nc.scalar.activation(x5[:], psF[:], ACTF.Relu)

        x5T = smp.tile([128, 2, 32], F32R, tag="x5T")
        for kb in range(2):
            trF = pcv.tile([128, 32], F32, tag="pc", name="trF")
            nc.tensor.transpose(trF[:], x5[:, 128 * kb:128 * (kb + 1)],
                                ct["c_ident"][0:32, 0:32])
            nc.vector.tensor_copy(x5T[:, kb, :], trF[:])

        psG = pcv.tile([32, 24], F32, tag="pc", name="psG")
        nc.tensor.matmul(psG[:], ones32[:], ct["c_fb2"][:],
                         start=True, stop=False)
        nc.tensor.matmul(psG[:], x5T[:, 0, :], ct["c_fw2"][:, 0, :],
                         start=False, stop=False)
        nc.tensor.matmul(psG[:], x5T[:, 1, :], ct["c_fw2"][:, 1, :],
                         start=False, stop=True)
        osb = smp.tile([32, 24], F32, tag="osb")
        nc.scalar.copy(osb[:, 0:2], psG[:, 0:2])
        nc.scalar.activation(osb[:, 2:4], psG[:, 2:4], ACTF.Sigmoid)
        nc.scalar.copy(osb[:, 4:24], psG[:, 4:24])
        nc.sync.dma_start(out=out_d, in_=osb[:])

    nc.compile()
    return nc


# ------------------------------------------------------------------ entry

_CACHE = {}


def _get_nc():
    if "nc" not in _CACHE:
        _CACHE["nc"] = build_nc()
    return _CACHE["nc"]


def make_in_maps(**inputs):
    consts = _build_consts(
        inputs["w1"], inputs["b1"], inputs["w2"], inputs["b2"],
        inputs["w3"], inputs["b3"], inputs["w4"], inputs["b4"],
        inputs["fw1"], inputs["fb1"], inputs["fw2"], inputs["fb2"],
    )
    image = np.ascontiguousarray(np.asarray(inputs["image"], np.float32))
    features = np.ascontiguousarray(np.asarray(inputs["features"], np.float32))
    centers = np.ascontiguousarray(np.asarray(inputs["centers"], np.float32))
    in_maps = []
    for i in range(NCORES):
        sl = slice(i * BL, (i + 1) * BL)
        m = {
            "image": np.ascontiguousarray(image[sl]),
            "features": np.ascontiguousarray(features[sl]),
            "centers": np.ascontiguousarray(centers[sl]),
        }
        m.update(consts)
        in_maps.append(m)
    return in_maps


def kernel(**inputs):
    nc = _get_nc()
    in_maps = make_in_maps(**inputs)
    res = run_bass_kernel_spmd(nc, in_maps, core_ids=list(range(NCORES)))
    out = np.concatenate([res.results[i]["out"] for i in range(NCORES)], axis=0)
    return out.astype(np.float32)

